# revision 1
# baseline (speedup 1.0000x reference)
"""Trainium2 Bass kernel for CausalWaveletFieldAttention.

Full-input contract: kernel(**inputs) takes the complete (unsharded) numpy
inputs and returns the full [8, 2048, 1024] float32 output.

Sharding: pure data-parallel over batch B=8 -> one batch element per
NeuronCore (8 cores), zero collectives (the head-coupling einsum mixes heads
within a batch element only).

Per-core pipeline (x pre-transposed to feature-major on host, bf16 compute,
fp32 PSUM accumulation):
  1. k = x @ Wk.T       (TensorE, moving operand = x columns)
  2. k2 = Square(k + bk) (ScalarE), per-head sums of 64 partitions via a
     selector matmul (TensorE) -> kmag = sqrt(.) (ScalarE)
  3. v = x @ Wv.T with v output channels permuted to d-major order
     (channel c~ = d*16 + h); field = (v + bv) * kmag  (fused DVE op)
  4. causal multi-scale dilated conv: collapsed to 22 distinct time
     offsets; each offset is one fused scalar_tensor_tensor MAC per
     channel tile on VectorE with per-partition (per-head) coefficients
  5. head coupling: in d-major layout the coupling matrix is
     I_8 (x) C^T -> a single block-diagonal [128,128] stationary matmul
     per channel tile (8x cheaper than a dense 1024x1024 coupling)
  6. gate = Sigmoid(x @ Wg.T + 2.0) (d-major), gated = z * gate
  7. out = gated.T @ Wo.T + out_b with gated [c~,n] chunks as the
     stationary operand so the output lands token-major for the DMA out.
The tiny softmaxes (scale gains [11,16], coupling [16,16]) are computed
on-device.
"""

import os
import sys

import numpy as np

# recover wedged NeuronCores from a previously killed process
os.environ.setdefault("NEURON_RT_RESET_CORES", "1")

for _p in ("/opt/trn_rl_repo", "/root/.axon_site/_ro/trn_rl_repo"):
    if _p not in sys.path:
        sys.path.append(_p)

import ml_dtypes  # noqa: E402
import concourse.bass as bass  # noqa: E402
import concourse.tile as tile  # noqa: E402
from concourse import bacc, mybir  # noqa: E402
from concourse import bass_utils  # noqa: E402

BF16 = mybir.dt.bfloat16
F32 = mybir.dt.float32
FP8 = mybir.dt.float8e4
NP_BF16 = ml_dtypes.bfloat16
NP_FP8 = ml_dtypes.float8_e4m3

B, N, D = 8, 2048, 1024
H, HD = 16, 64
S = 11  # scales
NCORES = 8
P = 128  # partitions
CH = D // P  # 8 channel chunks
NT = N // P  # 16 token tiles
NCK = N // 512  # 4 free-dim 512 chunks

D4 = np.array(
    [0.4829629131445341, 0.8365163037378079, 0.2241438680420134, -0.1294095225512604],
    dtype=np.float64,
)

# Distinct causal time offsets (3-t)*2^j < N, and the [n_offsets, S] map s.t.
# w[o, h] = sum_j A_MAP[o, j] * softmax_gains[j, h]
_offs = sorted({(3 - t) * (1 << j) for j in range(S) for t in range(4)} & set(range(N)))
OFFSETS = list(_offs)
NOFF = len(OFFSETS)  # 22
A_MAP = np.zeros((NOFF, S), dtype=np.float64)
for j in range(S):
    for t in range(4):
        o = (3 - t) * (1 << j)
        if o < N:
            A_MAP[OFFSETS.index(o), j] += D4[t]

# d-major channel permutation: c~ -> original feature h*64 + d
PERM = np.array([(c % H) * HD + c // H for c in range(D)], dtype=np.int64)

# conv offsets computed on TensorE (diag-stationary matmuls accumulated in
# PSUM) vs GpSimd vs VectorE (fused per-partition MACs); small offsets carry
# the most work, so shifting them to the PE balances the engines.
PE_SET = (0, 1, 2, 3, 4, 6, 12, 16, 24, 32)
POOL_SET = ()
PE_OFFS = [oi for oi, o in enumerate(OFFSETS) if o in PE_SET]
POOL_OFFS = [oi for oi, o in enumerate(OFFSETS) if o in POOL_SET]
DVE_OFFS = [oi for oi in range(NOFF)
            if oi not in PE_OFFS and oi not in POOL_OFFS]

_CACHE = {}


def _build_program(iters=1, ob_zero=False):
    nc = bacc.Bacc("TRN2", target_bir_lowering=False, debug=False, num_devices=NCORES)

    # ---- I/O ----
    x_cm = nc.dram_tensor("x_cm", [D, N], BF16, kind="ExternalInput")
    # fp8 DoubleRow operands for the k projection: contraction index
    # c = 256*ic + 2*ki + j laid out as [ki, ic, j, .]
    x8_d = nc.dram_tensor("x8", [P, 4, 2, N], FP8, kind="ExternalInput")
    wk8_d = nc.dram_tensor("wk8", [P, 4, 2, D], FP8, kind="ExternalInput")
    wk_d = nc.dram_tensor("wk", [D, D], BF16, kind="ExternalInput")  # [c_in, kf]
    wv_d = nc.dram_tensor("wv", [D, D], BF16, kind="ExternalInput")  # [c_in, c~]
    wg_d = nc.dram_tensor("wg", [D, D], BF16, kind="ExternalInput")  # [c_in, c~]
    wo_d = nc.dram_tensor("wo", [D, D], BF16, kind="ExternalInput")  # [c~, f]
    bk_d = nc.dram_tensor("bk", [P, CH], F32, kind="ExternalInput")
    bv_d = nc.dram_tensor("bv", [P, CH], F32, kind="ExternalInput")
    bg_d = nc.dram_tensor("bg", [P, CH], F32, kind="ExternalInput")
    ob_d = nc.dram_tensor("ob", [P, D], F32, kind="ExternalInput")  # out_b row-bcast
    sg_d = nc.dram_tensor("sg", [S, H], F32, kind="ExternalInput")
    fc_d = nc.dram_tensor("fc", [H, H], F32, kind="ExternalInput")
    y_d = nc.dram_tensor("y", [N, D], F32, kind="ExternalOutput")

    # ---- constants (embedded in NEFF) ----
    a_rep = np.zeros((H, S, NOFF), dtype=np.float32)
    for hh in range(H):
        a_rep[hh] = A_MAP.T.astype(np.float32)
    a_rep_d = nc.inline_tensor(np.ascontiguousarray(a_rep), "a_rep")
    sel = np.zeros((P, CH, H), dtype=NP_BF16)
    for kc in range(CH):
        for p in range(P):
            sel[p, kc, 2 * kc + p // HD] = 1
    sel_d = nc.inline_tensor(np.ascontiguousarray(sel), "sel")
    i16_d = nc.inline_tensor(np.eye(H, dtype=NP_BF16), "i16")
    i128_d = nc.inline_tensor(np.eye(P, dtype=NP_BF16), "i128")

    import contextlib
    with tile.TileContext(nc) as tc, contextlib.ExitStack() as _st:
      for _it in range(iters):
          with (
              tc.tile_pool(name="consts", bufs=1) as cpool,
              tc.tile_pool(name="xpool", bufs=1) as xpool,
              tc.tile_pool(name="wpool", bufs=2) as wpool,
              tc.tile_pool(name="field", bufs=1) as fpool,
              tc.tile_pool(name="accp", bufs=1) as apool,
              tc.tile_pool(name="gatep", bufs=1) as gpool,
              tc.tile_pool(name="k2p", bufs=3) as k2pool,
              tc.tile_pool(name="ystg", bufs=3) as ypool,
              tc.tile_pool(name="psum", bufs=4, space="PSUM") as pspool,
              tc.tile_pool(name="psum_km", bufs=2, space="PSUM") as kmpool,
          ):
              # ============ big streaming inputs first (head latency) ======
              x_sb = xpool.tile([P, CH, N], BF16)
              x8_sb = wpool.tile([P, 4, 2, N], FP8, tag="wmat")
              wk8_sb = wpool.tile([P, 4, 2, D], FP8, tag="wmat")
              nc.sync.dma_start(out=wk8_sb[:, :, :, :], in_=wk8_d[:, :, :, :])
              nc.sync.dma_start(out=x8_sb[:, :, :, :], in_=x8_d[:, :, :, :])
              for ic in range(CH):
                  nc.sync.dma_start(out=x_sb[:, ic, :], in_=x_cm[P * ic:P * (ic + 1), :])

              # ============ tiny parameter prep ============
              # softmax of scale_gain over scales, per head -> gains [16, 11]
              sg_sb = cpool.tile([H, S], F32)
              nc.gpsimd.dma_start(out=sg_sb[:, :], in_=sg_d.ap().rearrange("j h -> h j"))
              sg_mx = cpool.tile([H, 1], F32)
              nc.vector.reduce_max(out=sg_mx[:, :], in_=sg_sb[:, :], axis=mybir.AxisListType.X)
              nc.vector.tensor_scalar_mul(sg_mx[:, :], sg_mx[:, :], -1.0)
              sg_e = cpool.tile([H, S], F32)
              nc.scalar.activation(
                  out=sg_e[:, :], in_=sg_sb[:, :],
                  func=mybir.ActivationFunctionType.Exp, bias=sg_mx[:, 0:1], scale=1.0,
              )
              sg_sum = cpool.tile([H, 1], F32)
              nc.vector.reduce_sum(out=sg_sum[:, :], in_=sg_e[:, :], axis=mybir.AxisListType.X)
              sg_rec = cpool.tile([H, 1], F32)
              nc.vector.reciprocal(out=sg_rec[:, :], in_=sg_sum[:, :])
              gains = cpool.tile([H, S], F32)
              nc.vector.tensor_scalar_mul(gains[:, :], sg_e[:, :], sg_rec[:, 0:1])

              # conv coefficients w[h, o] = sum_j gains[h, j] * A_MAP[o, j]
              a_sb = cpool.tile([H, S, NOFF], F32)
              nc.gpsimd.dma_start(out=a_sb[:, :, :], in_=a_rep_d[:, :, :])
              w_sb = cpool.tile([H, NOFF], F32)
              nc.vector.tensor_scalar_mul(w_sb[:, :], a_sb[:, 0, :], gains[:, 0:1])
              for j in range(1, S):
                  nc.vector.scalar_tensor_tensor(
                      out=w_sb[:, :], in0=a_sb[:, j, :], scalar=gains[:, j:j + 1],
                      in1=w_sb[:, :], op0=mybir.AluOpType.mult, op1=mybir.AluOpType.add,
                  )
              # replicate to all 128 partitions (p -> p mod 16)
              w_rep = cpool.tile([P, NOFF], F32)
              for r in range(P // H):
                  nc.gpsimd.dma_start(out=w_rep[H * r:H * (r + 1), :], in_=w_sb[:, :])
              # diagonal stationary matrices for the PE-side conv offsets
              i128_sb = cpool.tile([P, P], BF16)
              nc.gpsimd.dma_start(out=i128_sb[:, :], in_=i128_d[:, :])
              gdiag = cpool.tile([P, len(PE_OFFS), P], BF16)
              for gi, oi in enumerate(PE_OFFS):
                  nc.vector.tensor_scalar_mul(
                      gdiag[:, gi, :], i128_sb[:, :], w_rep[:, oi:oi + 1]
                  )

              # coupling softmax (rows) -> C_sm; G = I_8 (x) C_sm^T  [128,128] bf16
              fc_sb = cpool.tile([H, H], F32)
              nc.gpsimd.dma_start(out=fc_sb[:, :], in_=fc_d[:, :])
              fc_mx = cpool.tile([H, 1], F32)
              nc.vector.reduce_max(out=fc_mx[:, :], in_=fc_sb[:, :], axis=mybir.AxisListType.X)
              nc.vector.tensor_scalar_mul(fc_mx[:, :], fc_mx[:, :], -1.0)
              fc_e = cpool.tile([H, H], F32)
              nc.scalar.activation(
                  out=fc_e[:, :], in_=fc_sb[:, :],
                  func=mybir.ActivationFunctionType.Exp, bias=fc_mx[:, 0:1], scale=1.0,
              )
              fc_sum = cpool.tile([H, 1], F32)
              nc.vector.reduce_sum(out=fc_sum[:, :], in_=fc_e[:, :], axis=mybir.AxisListType.X)
              fc_rec = cpool.tile([H, 1], F32)
              nc.vector.reciprocal(out=fc_rec[:, :], in_=fc_sum[:, :])
              csm_bf = cpool.tile([H, H], BF16)
              nc.vector.tensor_scalar_mul(csm_bf[:, :], fc_e[:, :], fc_rec[:, 0:1])
              i16_sb = cpool.tile([H, H], BF16)
              nc.gpsimd.dma_start(out=i16_sb[:, :], in_=i16_d[:, :])
              ct_ps = pspool.tile([H, H], BF16, tag="mm")
              nc.tensor.transpose(out=ct_ps[:, :], in_=csm_bf[:, :], identity=i16_sb[:, :])
              ct_bf = cpool.tile([H, H], BF16)
              nc.vector.tensor_copy(ct_bf[:, :], ct_ps[:, :])
              g_sb = cpool.tile([P, P], BF16)
              nc.vector.memset(g_sb[:, :], 0.0)
              for r in range(CH):
                  nc.sync.dma_start(
                      out=g_sb[H * r:H * (r + 1), H * r:H * (r + 1)], in_=ct_bf[:, :]
                  )

              sel_sb = cpool.tile([P, CH, H], BF16)
              nc.gpsimd.dma_start(out=sel_sb[:, :, :], in_=sel_d[:, :, :])
              bk_sb = cpool.tile([P, CH], F32)
              nc.gpsimd.dma_start(out=bk_sb[:, :], in_=bk_d[:, :])
              bv_sb = cpool.tile([P, CH], F32)
              nc.gpsimd.dma_start(out=bv_sb[:, :], in_=bv_d[:, :])
              bg_sb = cpool.tile([P, CH], F32)
              nc.gpsimd.dma_start(out=bg_sb[:, :], in_=bg_d[:, :])
              ob_sb = cpool.tile([P, D], F32)
              nc.gpsimd.dma_start(out=ob_sb[:, :], in_=ob_d[:, :])

              # ============ k phase: kmag[h, n] ============
              kmag16 = cpool.tile([H, N], BF16)
              for nch in range(NCK):
                  ns = 512 * nch
                  km_ps = kmpool.tile([H, 512], F32, tag="km")
                  for kc in range(CH):
                      ps = pspool.tile([P, 512], F32, tag="mm")
                      for ic in range(4):
                          nc.tensor.matmul(
                              ps[:, :],
                              lhsT=wk8_sb[:, ic, :, P * kc:P * (kc + 1)],
                              rhs=x8_sb[:, ic, :, ns:ns + 512],
                              perf_mode=mybir.MatmulPerfMode.DoubleRow,
                              start=(ic == 0), stop=(ic == 3),
                          )
                      k2 = k2pool.tile([P, 512], BF16, tag="k2")
                      nc.scalar.activation(
                          out=k2[:, :], in_=ps[:, :],
                          func=mybir.ActivationFunctionType.Square,
                          bias=bk_sb[:, kc:kc + 1], scale=1.0,
                      )
                      nc.tensor.matmul(
                          km_ps[:, :],
                          lhsT=sel_sb[:, kc, :], rhs=k2[:, :],
                          start=(kc == 0), stop=(kc == CH - 1),
                      )
                  nc.scalar.activation(
                      out=kmag16[:, ns:ns + 512], in_=km_ps[:, :],
                      func=mybir.ActivationFunctionType.Sqrt,
                  )
              kmag_rep = cpool.tile([P, N], BF16)
              for r in range(P // H):
                  nc.sync.dma_start(out=kmag_rep[H * r:H * (r + 1), :], in_=kmag16[:, :])

              # ============ v phase + conv stages A/B-seg0, interleaved per vc ==
              wv_sb = wpool.tile([P, CH, D], BF16, tag="wmat")
              for ic in range(CH):
                  nc.sync.dma_start(out=wv_sb[:, ic, :], in_=wv_d[P * ic:P * (ic + 1), :])
              field = fpool.tile([P, CH, N], BF16)
              acc = apool.tile([P, CH, N], BF16)
              HN = N // 2
              for vc in range(CH):
                  for nch in range(NCK):
                      ns = 512 * nch
                      ps = pspool.tile([P, 512], F32, tag="mm")
                      for ic in range(CH):
                          nc.tensor.matmul(
                              ps[:, :],
                              lhsT=wv_sb[:, ic, P * vc:P * (vc + 1)],
                              rhs=x_sb[:, ic, ns:ns + 512],
                              start=(ic == 0), stop=(ic == CH - 1),
                          )
                      nc.vector.scalar_tensor_tensor(
                          out=field[:, vc, ns:ns + 512],
                          in0=ps[:, :], scalar=bv_sb[:, vc:vc + 1],
                          in1=kmag_rep[:, ns:ns + 512],
                          op0=mybir.AluOpType.add, op1=mybir.AluOpType.mult,
                      )
                  # conv stage A for this vc: PE offsets -> PSUM, first DVE
                  # MAC folds the partial into acc
                  for nch in range(NCK):
                      ns = 512 * nch
                      psc = pspool.tile([P, 512], F32, tag="mm")
                      for k_i, oi in enumerate(PE_OFFS):
                          o = OFFSETS[oi]
                          lo = max(0, o - ns)  # first out col in this chunk
                          # k_i == 0 is offset 0 (full width) so start=True
                          # always covers every psum column.
                          nc.tensor.matmul(
                              psc[:, lo:512],
                              lhsT=gdiag[:, k_i, :],
                              rhs=field[:, vc, ns + lo - o:ns + 512 - o],
                              start=(k_i == 0), stop=(k_i == len(PE_OFFS) - 1),
                          )
                      oi0 = DVE_OFFS[0]
                      o0 = OFFSETS[oi0]
                      lo0 = max(0, o0 - ns)
                      if lo0 > 0:
                          nc.vector.tensor_copy(acc[:, vc, ns:ns + lo0], psc[:, 0:lo0])
                      nc.vector.scalar_tensor_tensor(
                          out=acc[:, vc, ns + lo0:ns + 512],
                          in0=field[:, vc, ns + lo0 - o0:ns + 512 - o0],
                          scalar=w_rep[:, oi0:oi0 + 1],
                          in1=psc[:, lo0:512],
                          op0=mybir.AluOpType.mult, op1=mybir.AluOpType.add,
                      )
                  # conv stage B segment 0 for this vc (cols [o, 512))
                  for oi in DVE_OFFS[1:]:
                      o = OFFSETS[oi]
                      if o < 512:
                          nc.vector.scalar_tensor_tensor(
                              out=acc[:, vc, o:512],
                              in0=field[:, vc, 0:512 - o],
                              scalar=w_rep[:, oi:oi + 1],
                              in1=acc[:, vc, o:512],
                              op0=mybir.AluOpType.mult, op1=mybir.AluOpType.add,
                          )

              # ============ gate phase (d-major channels) ============
              wg_sb = wpool.tile([P, CH, D], BF16, tag="wmat")
              for ic in range(CH):
                  nc.sync.dma_start(out=wg_sb[:, ic, :], in_=wg_d[P * ic:P * (ic + 1), :])
              gate = gpool.tile([P, CH, N], BF16)
              for gc in range(CH):
                  for nch in range(NCK):
                      ns = 512 * nch
                      ps = pspool.tile([P, 512], F32, tag="mm")
                      for ic in range(CH):
                          nc.tensor.matmul(
                              ps[:, :],
                              lhsT=wg_sb[:, ic, P * gc:P * (gc + 1)],
                              rhs=x_sb[:, ic, ns:ns + 512],
                              start=(ic == 0), stop=(ic == CH - 1),
                          )
                      nc.scalar.activation(
                          out=gate[:, gc, ns:ns + 512], in_=ps[:, :],
                          func=mybir.ActivationFunctionType.Sigmoid,
                          bias=bg_sb[:, gc:gc + 1], scale=1.0,
                      )

              wo_sb = wpool.tile([P, CH, D], BF16, tag="wmat")
              for ic in range(CH):
                  nc.sync.dma_start(out=wo_sb[:, ic, :], in_=wo_d[P * ic:P * (ic + 1), :])

              # per 512-col segment: finish conv stage B, then coupling +
              # gate-mul (gated reuses the x buffer, dead after the gate
              # matmuls), then the out projection for that segment's tokens
              gated = x_sb
              for seg in range(NCK):
                  ns = 512 * seg
                  if seg >= 1:
                      for vc in range(CH):
                          for oi in DVE_OFFS[1:]:
                              o = OFFSETS[oi]
                              lo = max(ns, o)
                              if lo < ns + 512:
                                  nc.vector.scalar_tensor_tensor(
                                      out=acc[:, vc, lo:ns + 512],
                                      in0=field[:, vc, lo - o:ns + 512 - o],
                                      scalar=w_rep[:, oi:oi + 1],
                                      in1=acc[:, vc, lo:ns + 512],
                                      op0=mybir.AluOpType.mult,
                                      op1=mybir.AluOpType.add,
                                  )
                  for vc in range(CH):
                      ps = pspool.tile([P, 512], F32, tag="mm")
                      nc.tensor.matmul(
                          ps[:, :], lhsT=g_sb[:, :], rhs=acc[:, vc, ns:ns + 512],
                          start=True, stop=True,
                      )
                      nc.vector.tensor_mul(
                          gated[:, vc, ns:ns + 512], ps[:, :],
                          gate[:, vc, ns:ns + 512],
                      )
                  # ============ out projection for this segment ============
                  for nt in range(4 * seg, 4 * seg + 4):
                      ystg = ypool.tile([P, D], F32, tag="y")
                      for fch in range(2):
                          fs = 512 * fch
                          ps = pspool.tile([P, 512], F32, tag="mm")
                          for vc in range(CH):
                              nc.tensor.matmul(
                                  ps[:, :],
                                  lhsT=gated[:, vc, P * nt:P * (nt + 1)],
                                  rhs=wo_sb[:, vc, fs:fs + 512],
                                  start=(vc == 0), stop=(vc == CH - 1),
                              )
                          if ob_zero:
                              nc.scalar.activation(
                                  out=ystg[:, fs:fs + 512], in_=ps[:, :],
                                  func=mybir.ActivationFunctionType.Copy,
                              )
                          else:
                              nc.vector.tensor_add(
                                  ystg[:, fs:fs + 512], ps[:, :],
                                  ob_sb[:, fs:fs + 512],
                              )
                      nc.sync.dma_start(out=y_d[P * nt:P * (nt + 1), :], in_=ystg[:, :])


    nc.compile()
    return nc


def _prep_shared(qkv_w, qkv_b, out_w, out_b, gate_w, gate_b, scale_gain, field_coupling):
    perm = PERM
    wk = np.ascontiguousarray(qkv_w[D:2 * D, :].T.astype(NP_BF16))
    wk8 = np.ascontiguousarray(
        qkv_w[D:2 * D, :].T.reshape(4, P, 2, D).transpose(1, 0, 2, 3)
        .astype(NP_FP8))
    wv = np.ascontiguousarray(qkv_w[2 * D:3 * D, :][perm, :].T.astype(NP_BF16))
    wg = np.ascontiguousarray(gate_w[perm, :].T.astype(NP_BF16))
    wo = np.ascontiguousarray(out_w[:, perm].T.astype(NP_BF16))
    bk = np.ascontiguousarray(qkv_b[D:2 * D].reshape(CH, P).T.astype(np.float32))
    bv = np.ascontiguousarray(qkv_b[2 * D:3 * D][perm].reshape(CH, P).T.astype(np.float32))
    bg = np.ascontiguousarray(gate_b[perm].reshape(CH, P).T.astype(np.float32))
    ob = np.ascontiguousarray(np.broadcast_to(out_b.astype(np.float32), (P, D)))
    sg = np.ascontiguousarray(scale_gain.astype(np.float32))
    fc = np.ascontiguousarray(field_coupling.astype(np.float32))
    return {"wk": wk, "wk8": wk8, "wv": wv, "wg": wg, "wo": wo, "bk": bk,
            "bv": bv, "bg": bg, "ob": ob, "sg": sg, "fc": fc}


def _make_in_maps(x, shared):
    in_maps = []
    for b in range(B):
        m = dict(shared)
        xt = x[b].T
        m["x_cm"] = np.ascontiguousarray(xt.astype(NP_BF16))
        m["x8"] = np.ascontiguousarray(
            xt.reshape(4, P, 2, N).transpose(1, 0, 2, 3).astype(NP_FP8))
        in_maps.append(m)
    return in_maps


def kernel(x, qkv_w, qkv_b, out_w, out_b, gate_w, gate_b, scale_gain,
           field_coupling):
    x = np.asarray(x, dtype=np.float32)
    qkv_w = np.asarray(qkv_w, dtype=np.float32)
    qkv_b = np.asarray(qkv_b, dtype=np.float32)
    out_w = np.asarray(out_w, dtype=np.float32)
    out_b = np.asarray(out_b, dtype=np.float32)
    gate_w = np.asarray(gate_w, dtype=np.float32)
    gate_b = np.asarray(gate_b, dtype=np.float32)
    scale_gain = np.asarray(scale_gain, dtype=np.float32)
    field_coupling = np.asarray(field_coupling, dtype=np.float32)

    ob_zero = not np.any(out_b)
    key = ("nc", ob_zero)
    if key not in _CACHE:
        _CACHE[key] = _build_program(ob_zero=ob_zero)
    nc = _CACHE[key]

    shared = _prep_shared(qkv_w, qkv_b, out_w, out_b, gate_w, gate_b,
                          scale_gain, field_coupling)
    in_maps = _make_in_maps(x, shared)

    res = bass_utils.run_bass_kernel_spmd(nc, in_maps, list(range(NCORES)))
    out = np.stack([np.asarray(res.results[b]["y"], dtype=np.float32)
                    for b in range(B)], axis=0)
    return out



# revision 12
# speedup vs baseline: 1.2790x; 1.2790x over previous
"""Trainium2 Bass kernel for CausalWaveletFieldAttention.

Full-input contract: kernel(**inputs) takes the complete (unsharded) numpy
inputs and returns the full [8, 2048, 1024] float32 output.

Sharding: pure data-parallel over batch B=8 -> one batch element per
NeuronCore (8 cores), zero collectives (the head-coupling einsum mixes heads
within a batch element only).

Per-core pipeline (x pre-transposed to feature-major on host, bf16 compute,
fp32 PSUM accumulation):
  1. k = x @ Wk.T       (TensorE, moving operand = x columns)
  2. k2 = Square(k + bk) (ScalarE), per-head sums of 64 partitions via a
     selector matmul (TensorE) -> kmag = sqrt(.) (ScalarE)
  3. v = x @ Wv.T with v output channels permuted to d-major order
     (channel c~ = d*16 + h); field = (v + bv) * kmag  (fused DVE op)
  4. causal multi-scale dilated conv: collapsed to 22 distinct time
     offsets; each offset is one fused scalar_tensor_tensor MAC per
     channel tile on VectorE with per-partition (per-head) coefficients
  5. head coupling: in d-major layout the coupling matrix is
     I_8 (x) C^T -> a single block-diagonal [128,128] stationary matmul
     per channel tile (8x cheaper than a dense 1024x1024 coupling)
  6. gate = Sigmoid(x @ Wg.T + 2.0) (d-major), gated = z * gate
  7. out = gated.T @ Wo.T + out_b with gated [c~,n] chunks as the
     stationary operand so the output lands token-major for the DMA out.
The tiny softmaxes (scale gains [11,16], coupling [16,16]) are computed
on-device.
"""

import os
import sys

import numpy as np

# recover wedged NeuronCores from a previously killed process
os.environ.setdefault("NEURON_RT_RESET_CORES", "1")

for _p in ("/opt/trn_rl_repo", "/root/.axon_site/_ro/trn_rl_repo"):
    if _p not in sys.path:
        sys.path.append(_p)

import ml_dtypes  # noqa: E402
import concourse.bass as bass  # noqa: E402
import concourse.tile as tile  # noqa: E402
from concourse import bacc, mybir  # noqa: E402
from concourse import bass_utils  # noqa: E402

BF16 = mybir.dt.bfloat16
F32 = mybir.dt.float32
FP8 = mybir.dt.float8e4
NP_BF16 = ml_dtypes.bfloat16
NP_FP8 = ml_dtypes.float8_e4m3

B, N, D = 8, 2048, 1024
H, HD = 16, 64
S = 11  # scales
NCORES = 8
P = 128  # partitions
CH = D // P  # 8 channel chunks
NT = N // P  # 16 token tiles
NCK = N // 512  # 4 free-dim 512 chunks

D4 = np.array(
    [0.4829629131445341, 0.8365163037378079, 0.2241438680420134, -0.1294095225512604],
    dtype=np.float64,
)

# Distinct causal time offsets (3-t)*2^j < N, and the [n_offsets, S] map s.t.
# w[o, h] = sum_j A_MAP[o, j] * softmax_gains[j, h]
_offs = sorted({(3 - t) * (1 << j) for j in range(S) for t in range(4)} & set(range(N)))
OFFSETS = list(_offs)
NOFF = len(OFFSETS)  # 22
A_MAP = np.zeros((NOFF, S), dtype=np.float64)
for j in range(S):
    for t in range(4):
        o = (3 - t) * (1 << j)
        if o < N:
            A_MAP[OFFSETS.index(o), j] += D4[t]

# d-major channel permutation: c~ -> original feature h*64 + d
PERM = np.array([(c % H) * HD + c // H for c in range(D)], dtype=np.int64)

# conv offsets computed on TensorE (diag-stationary matmuls accumulated in
# PSUM) vs GpSimd vs VectorE (fused per-partition MACs); small offsets carry
# the most work, so shifting them to the PE balances the engines.
PE_SET = (0, 1, 2, 3, 4, 6, 12, 16, 24, 32)
POOL_SET = ()
PE_OFFS = [oi for oi, o in enumerate(OFFSETS) if o in PE_SET]
POOL_OFFS = [oi for oi, o in enumerate(OFFSETS) if o in POOL_SET]
DVE_OFFS = [oi for oi in range(NOFF)
            if oi not in PE_OFFS and oi not in POOL_OFFS]

_CACHE = {}


def _build_program(iters=1, ob_zero=False):
    nc = bacc.Bacc("TRN2", target_bir_lowering=False, debug=False, num_devices=NCORES)

    # ---- I/O ----
    x_cm = nc.dram_tensor("x_cm", [D, N], BF16, kind="ExternalInput")
    # fp8 DoubleRow operands for the k projection: contraction index
    # c = 256*ic + 2*ki + j laid out as [ki, ic, j, .]
    x8_d = nc.dram_tensor("x8", [P, 4, 2, N], FP8, kind="ExternalInput")
    wk8_d = nc.dram_tensor("wk8", [P, 4, 2, D], FP8, kind="ExternalInput")
    wv_d = nc.dram_tensor("wv", [D, D], BF16, kind="ExternalInput")  # [c_in, c~]
    wg8_d = nc.dram_tensor("wg8", [P, 4, 2, D], FP8, kind="ExternalInput")
    wo_d = nc.dram_tensor("wo", [D, D], BF16, kind="ExternalInput")  # [c~, f]
    bk_d = nc.dram_tensor("bk", [P, CH], F32, kind="ExternalInput")
    bv_d = nc.dram_tensor("bv", [P, CH], F32, kind="ExternalInput")
    bg_d = nc.dram_tensor("bg", [P, CH], F32, kind="ExternalInput")
    ob_d = nc.dram_tensor("ob", [P, D], F32, kind="ExternalInput")  # out_b row-bcast
    sg_d = nc.dram_tensor("sg", [S, H], F32, kind="ExternalInput")
    fc_d = nc.dram_tensor("fc", [H, H], F32, kind="ExternalInput")
    y_d = nc.dram_tensor("y", [N, D], BF16, kind="ExternalOutput")

    # ---- constants (embedded in NEFF) ----
    a_rep = np.zeros((H, S, NOFF), dtype=np.float32)
    for hh in range(H):
        a_rep[hh] = A_MAP.T.astype(np.float32)
    a_rep_d = nc.inline_tensor(np.ascontiguousarray(a_rep), "a_rep")
    sel = np.zeros((P, CH, H), dtype=NP_BF16)
    for kc in range(CH):
        for p in range(P):
            sel[p, kc, 2 * kc + p // HD] = 1
    sel_d = nc.inline_tensor(np.ascontiguousarray(sel), "sel")
    i16_d = nc.inline_tensor(np.eye(H, dtype=NP_BF16), "i16")
    i128_d = nc.inline_tensor(np.eye(P, dtype=NP_BF16), "i128")

    import contextlib
    with tile.TileContext(nc) as tc, contextlib.ExitStack() as _st:
      for _it in range(iters):
          with (
              tc.tile_pool(name="consts", bufs=1) as cpool,
              tc.tile_pool(name="xpool", bufs=1) as xpool,
              tc.tile_pool(name="x8p", bufs=1) as x8pool,
              tc.tile_pool(name="wpool", bufs=2) as wpool,
              tc.tile_pool(name="field", bufs=1) as fpool,
              tc.tile_pool(name="accp", bufs=1) as apool,
              tc.tile_pool(name="gatep", bufs=1) as gpool,
              tc.tile_pool(name="k2p", bufs=3) as k2pool,
              tc.tile_pool(name="ystg", bufs=3) as ypool,
              tc.tile_pool(name="psum", bufs=4, space="PSUM") as pspool,
              tc.tile_pool(name="psum_km", bufs=2, space="PSUM") as kmpool,
          ):
              # ============ big streaming inputs first (head latency) ======
              x_sb = xpool.tile([P, CH, N], BF16)
              x8_sb = x8pool.tile([P, 4, 2, N], FP8)
              wk8_sb = wpool.tile([P, 4, 2, D], FP8, tag="wmat")
              nc.sync.dma_start(out=wk8_sb[:, :, :, :], in_=wk8_d[:, :, :, :])
              nc.sync.dma_start(out=x8_sb[:, :, :, :], in_=x8_d[:, :, :, :])
              for ic in range(CH):
                  nc.sync.dma_start(out=x_sb[:, ic, :], in_=x_cm[P * ic:P * (ic + 1), :])

              # ============ tiny parameter prep ============
              # softmax of scale_gain over scales, per head -> gains [16, 11]
              sg_sb = cpool.tile([H, S], F32)
              nc.gpsimd.dma_start(out=sg_sb[:, :], in_=sg_d.ap().rearrange("j h -> h j"))
              sg_mx = cpool.tile([H, 1], F32)
              nc.vector.reduce_max(out=sg_mx[:, :], in_=sg_sb[:, :], axis=mybir.AxisListType.X)
              nc.vector.tensor_scalar_mul(sg_mx[:, :], sg_mx[:, :], -1.0)
              sg_e = cpool.tile([H, S], F32)
              nc.scalar.activation(
                  out=sg_e[:, :], in_=sg_sb[:, :],
                  func=mybir.ActivationFunctionType.Exp, bias=sg_mx[:, 0:1], scale=1.0,
              )
              sg_sum = cpool.tile([H, 1], F32)
              nc.vector.reduce_sum(out=sg_sum[:, :], in_=sg_e[:, :], axis=mybir.AxisListType.X)
              sg_rec = cpool.tile([H, 1], F32)
              nc.vector.reciprocal(out=sg_rec[:, :], in_=sg_sum[:, :])
              gains = cpool.tile([H, S], F32)
              nc.vector.tensor_scalar_mul(gains[:, :], sg_e[:, :], sg_rec[:, 0:1])

              # conv coefficients w[h, o] = sum_j gains[h, j] * A_MAP[o, j]
              a_sb = cpool.tile([H, S, NOFF], F32)
              nc.gpsimd.dma_start(out=a_sb[:, :, :], in_=a_rep_d[:, :, :])
              w_sb = cpool.tile([H, NOFF], F32)
              nc.vector.tensor_scalar_mul(w_sb[:, :], a_sb[:, 0, :], gains[:, 0:1])
              for j in range(1, S):
                  nc.vector.scalar_tensor_tensor(
                      out=w_sb[:, :], in0=a_sb[:, j, :], scalar=gains[:, j:j + 1],
                      in1=w_sb[:, :], op0=mybir.AluOpType.mult, op1=mybir.AluOpType.add,
                  )
              # replicate to all 128 partitions (p -> p mod 16)
              w_rep = cpool.tile([P, NOFF], F32)
              for r in range(P // H):
                  nc.gpsimd.dma_start(out=w_rep[H * r:H * (r + 1), :], in_=w_sb[:, :])
              # diagonal stationary matrices for the PE-side conv offsets
              i128_sb = cpool.tile([P, P], BF16)
              nc.gpsimd.dma_start(out=i128_sb[:, :], in_=i128_d[:, :])
              gdiag = cpool.tile([P, len(PE_OFFS), P], BF16)
              for gi, oi in enumerate(PE_OFFS):
                  nc.vector.tensor_scalar_mul(
                      gdiag[:, gi, :], i128_sb[:, :], w_rep[:, oi:oi + 1]
                  )

              # coupling softmax (rows) -> C_sm; G = I_8 (x) C_sm^T  [128,128] bf16
              fc_sb = cpool.tile([H, H], F32)
              nc.gpsimd.dma_start(out=fc_sb[:, :], in_=fc_d[:, :])
              fc_mx = cpool.tile([H, 1], F32)
              nc.vector.reduce_max(out=fc_mx[:, :], in_=fc_sb[:, :], axis=mybir.AxisListType.X)
              nc.vector.tensor_scalar_mul(fc_mx[:, :], fc_mx[:, :], -1.0)
              fc_e = cpool.tile([H, H], F32)
              nc.scalar.activation(
                  out=fc_e[:, :], in_=fc_sb[:, :],
                  func=mybir.ActivationFunctionType.Exp, bias=fc_mx[:, 0:1], scale=1.0,
              )
              fc_sum = cpool.tile([H, 1], F32)
              nc.vector.reduce_sum(out=fc_sum[:, :], in_=fc_e[:, :], axis=mybir.AxisListType.X)
              fc_rec = cpool.tile([H, 1], F32)
              nc.vector.reciprocal(out=fc_rec[:, :], in_=fc_sum[:, :])
              csm_bf = cpool.tile([H, H], BF16)
              nc.vector.tensor_scalar_mul(csm_bf[:, :], fc_e[:, :], fc_rec[:, 0:1])
              i16_sb = cpool.tile([H, H], BF16)
              nc.gpsimd.dma_start(out=i16_sb[:, :], in_=i16_d[:, :])
              ct_ps = pspool.tile([H, H], BF16, tag="mm")
              nc.tensor.transpose(out=ct_ps[:, :], in_=csm_bf[:, :], identity=i16_sb[:, :])
              ct_bf = cpool.tile([H, H], BF16)
              nc.vector.tensor_copy(ct_bf[:, :], ct_ps[:, :])
              g_sb = cpool.tile([P, P], BF16)
              nc.vector.memset(g_sb[:, :], 0.0)
              for r in range(CH):
                  nc.sync.dma_start(
                      out=g_sb[H * r:H * (r + 1), H * r:H * (r + 1)], in_=ct_bf[:, :]
                  )

              sel_sb = cpool.tile([P, CH, H], BF16)
              nc.gpsimd.dma_start(out=sel_sb[:, :, :], in_=sel_d[:, :, :])
              bk_sb = cpool.tile([P, CH], F32)
              nc.gpsimd.dma_start(out=bk_sb[:, :], in_=bk_d[:, :])
              bv_sb = cpool.tile([P, CH], F32)
              nc.gpsimd.dma_start(out=bv_sb[:, :], in_=bv_d[:, :])
              bg_sb = cpool.tile([P, CH], F32)
              nc.gpsimd.dma_start(out=bg_sb[:, :], in_=bg_d[:, :])
              ob_sb = cpool.tile([P, D], F32)
              nc.gpsimd.dma_start(out=ob_sb[:, :], in_=ob_d[:, :])

              # ============ k phase: kmag[h, n] ============
              kmag16 = cpool.tile([H, N], BF16)
              for nch in range(NCK):
                  ns = 512 * nch
                  km_ps = kmpool.tile([H, 512], F32, tag="km")
                  for kc in range(CH):
                      ps = pspool.tile([P, 512], F32, tag="mm")
                      for ic in range(4):
                          nc.tensor.matmul(
                              ps[:, :],
                              lhsT=wk8_sb[:, ic, :, P * kc:P * (kc + 1)],
                              rhs=x8_sb[:, ic, :, ns:ns + 512],
                              perf_mode=mybir.MatmulPerfMode.DoubleRow,
                              start=(ic == 0), stop=(ic == 3),
                          )
                      k2 = k2pool.tile([P, 512], BF16, tag="k2")
                      nc.scalar.activation(
                          out=k2[:, :], in_=ps[:, :],
                          func=mybir.ActivationFunctionType.Square,
                          bias=bk_sb[:, kc:kc + 1], scale=1.0,
                      )
                      nc.tensor.matmul(
                          km_ps[:, :],
                          lhsT=sel_sb[:, kc, :], rhs=k2[:, :],
                          start=(kc == 0), stop=(kc == CH - 1),
                      )
                  nc.scalar.activation(
                      out=kmag16[:, ns:ns + 512], in_=km_ps[:, :],
                      func=mybir.ActivationFunctionType.Sqrt,
                  )
              kmag_rep = cpool.tile([P, N], BF16)
              for r in range(P // H):
                  nc.sync.dma_start(out=kmag_rep[H * r:H * (r + 1), :], in_=kmag16[:, :])

              # ============ v phase + conv stages A/B-seg0, interleaved per vc ==
              wv_sb = wpool.tile([P, CH, D], BF16, tag="wmat")
              for ic in range(CH):
                  nc.sync.dma_start(out=wv_sb[:, ic, :], in_=wv_d[P * ic:P * (ic + 1), :])
              field = fpool.tile([P, CH, N], BF16)
              acc = apool.tile([P, CH, N], BF16)
              HN = N // 2
              for vc in range(CH):
                  for nch in range(NCK):
                      ns = 512 * nch
                      ps = pspool.tile([P, 512], F32, tag="mm")
                      for ic in range(CH):
                          nc.tensor.matmul(
                              ps[:, :],
                              lhsT=wv_sb[:, ic, P * vc:P * (vc + 1)],
                              rhs=x_sb[:, ic, ns:ns + 512],
                              start=(ic == 0), stop=(ic == CH - 1),
                          )
                      nc.vector.scalar_tensor_tensor(
                          out=field[:, vc, ns:ns + 512],
                          in0=ps[:, :], scalar=bv_sb[:, vc:vc + 1],
                          in1=kmag_rep[:, ns:ns + 512],
                          op0=mybir.AluOpType.add, op1=mybir.AluOpType.mult,
                      )
                  # conv stage A for this vc: PE offsets -> PSUM, first DVE
                  # MAC folds the partial into acc
                  for nch in range(NCK):
                      ns = 512 * nch
                      psc = pspool.tile([P, 512], F32, tag="mm")
                      for k_i, oi in enumerate(PE_OFFS):
                          o = OFFSETS[oi]
                          lo = max(0, o - ns)  # first out col in this chunk
                          # k_i == 0 is offset 0 (full width) so start=True
                          # always covers every psum column.
                          nc.tensor.matmul(
                              psc[:, lo:512],
                              lhsT=gdiag[:, k_i, :],
                              rhs=field[:, vc, ns + lo - o:ns + 512 - o],
                              start=(k_i == 0), stop=(k_i == len(PE_OFFS) - 1),
                          )
                      oi0 = DVE_OFFS[0]
                      o0 = OFFSETS[oi0]
                      lo0 = max(0, o0 - ns)
                      if lo0 > 0:
                          nc.vector.tensor_copy(acc[:, vc, ns:ns + lo0], psc[:, 0:lo0])
                      nc.vector.scalar_tensor_tensor(
                          out=acc[:, vc, ns + lo0:ns + 512],
                          in0=field[:, vc, ns + lo0 - o0:ns + 512 - o0],
                          scalar=w_rep[:, oi0:oi0 + 1],
                          in1=psc[:, lo0:512],
                          op0=mybir.AluOpType.mult, op1=mybir.AluOpType.add,
                      )
                  # conv stage B segment 0 for this vc (cols [o, 512))
                  for oi in DVE_OFFS[1:]:
                      o = OFFSETS[oi]
                      if o < 512:
                          nc.vector.scalar_tensor_tensor(
                              out=acc[:, vc, o:512],
                              in0=field[:, vc, 0:512 - o],
                              scalar=w_rep[:, oi:oi + 1],
                              in1=acc[:, vc, o:512],
                              op0=mybir.AluOpType.mult, op1=mybir.AluOpType.add,
                          )

              # ============ gate phase (d-major channels, fp8 DoubleRow) ====
              wg8_sb = wpool.tile([P, 4, 2, D], FP8, tag="wmat")
              nc.sync.dma_start(out=wg8_sb[:, :, :, :], in_=wg8_d[:, :, :, :])
              gate = gpool.tile([P, CH, N], BF16)
              for gc in range(CH):
                  for nch in range(NCK):
                      ns = 512 * nch
                      ps = pspool.tile([P, 512], F32, tag="mm")
                      for ic in range(4):
                          nc.tensor.matmul(
                              ps[:, :],
                              lhsT=wg8_sb[:, ic, :, P * gc:P * (gc + 1)],
                              rhs=x8_sb[:, ic, :, ns:ns + 512],
                              perf_mode=mybir.MatmulPerfMode.DoubleRow,
                              start=(ic == 0), stop=(ic == 3),
                          )
                      nc.scalar.activation(
                          out=gate[:, gc, ns:ns + 512], in_=ps[:, :],
                          func=mybir.ActivationFunctionType.Sigmoid,
                          bias=bg_sb[:, gc:gc + 1], scale=1.0,
                      )

              wo_sb = wpool.tile([P, CH, D], BF16, tag="wmat")
              for ic in range(CH):
                  nc.sync.dma_start(out=wo_sb[:, ic, :], in_=wo_d[P * ic:P * (ic + 1), :])

              # per 512-col segment: finish conv stage B, then coupling +
              # gate-mul (gated reuses the x buffer, dead after the gate
              # matmuls), then the out projection for that segment's tokens
              gated = x_sb
              for seg in range(NCK):
                  ns = 512 * seg
                  if seg >= 1:
                      for vc in range(CH):
                          for oi in DVE_OFFS[1:]:
                              o = OFFSETS[oi]
                              lo = max(ns, o)
                              if lo < ns + 512:
                                  nc.vector.scalar_tensor_tensor(
                                      out=acc[:, vc, lo:ns + 512],
                                      in0=field[:, vc, lo - o:ns + 512 - o],
                                      scalar=w_rep[:, oi:oi + 1],
                                      in1=acc[:, vc, lo:ns + 512],
                                      op0=mybir.AluOpType.mult,
                                      op1=mybir.AluOpType.add,
                                  )
                  for vc in range(CH):
                      ps = pspool.tile([P, 512], F32, tag="mm")
                      nc.tensor.matmul(
                          ps[:, :], lhsT=g_sb[:, :], rhs=acc[:, vc, ns:ns + 512],
                          start=True, stop=True,
                      )
                      nc.vector.tensor_mul(
                          gated[:, vc, ns:ns + 512], ps[:, :],
                          gate[:, vc, ns:ns + 512],
                      )
                  # ============ out projection for this segment ============
                  for nt in range(4 * seg, 4 * seg + 4):
                      ystg = ypool.tile([P, D], BF16, tag="y")
                      for fch in range(2):
                          fs = 512 * fch
                          ps = pspool.tile([P, 512], F32, tag="mm")
                          for vc in range(CH):
                              nc.tensor.matmul(
                                  ps[:, :],
                                  lhsT=gated[:, vc, P * nt:P * (nt + 1)],
                                  rhs=wo_sb[:, vc, fs:fs + 512],
                                  start=(vc == 0), stop=(vc == CH - 1),
                              )
                          if ob_zero:
                              nc.scalar.activation(
                                  out=ystg[:, fs:fs + 512], in_=ps[:, :],
                                  func=mybir.ActivationFunctionType.Copy,
                              )
                          else:
                              nc.vector.tensor_add(
                                  ystg[:, fs:fs + 512], ps[:, :],
                                  ob_sb[:, fs:fs + 512],
                              )
                      nc.sync.dma_start(out=y_d[P * nt:P * (nt + 1), :], in_=ystg[:, :])


    nc.compile()
    return nc


def _prep_shared(qkv_w, qkv_b, out_w, out_b, gate_w, gate_b, scale_gain, field_coupling):
    perm = PERM
    wk8 = np.ascontiguousarray(
        qkv_w[D:2 * D, :].T.reshape(4, P, 2, D).transpose(1, 0, 2, 3)
        .astype(NP_FP8))
    wv = np.ascontiguousarray(qkv_w[2 * D:3 * D, :][perm, :].T.astype(NP_BF16))
    wg8 = np.ascontiguousarray(
        gate_w[perm, :].T.reshape(4, P, 2, D).transpose(1, 0, 2, 3)
        .astype(NP_FP8))
    wo = np.ascontiguousarray(out_w[:, perm].T.astype(NP_BF16))
    bk = np.ascontiguousarray(qkv_b[D:2 * D].reshape(CH, P).T.astype(np.float32))
    bv = np.ascontiguousarray(qkv_b[2 * D:3 * D][perm].reshape(CH, P).T.astype(np.float32))
    bg = np.ascontiguousarray(gate_b[perm].reshape(CH, P).T.astype(np.float32))
    ob = np.ascontiguousarray(np.broadcast_to(out_b.astype(np.float32), (P, D)))
    sg = np.ascontiguousarray(scale_gain.astype(np.float32))
    fc = np.ascontiguousarray(field_coupling.astype(np.float32))
    return {"wk8": wk8, "wv": wv, "wg8": wg8, "wo": wo, "bk": bk,
            "bv": bv, "bg": bg, "ob": ob, "sg": sg, "fc": fc}


def _make_in_maps(x, shared):
    in_maps = []
    for b in range(B):
        m = dict(shared)
        xt = x[b].T
        m["x_cm"] = np.ascontiguousarray(xt.astype(NP_BF16))
        m["x8"] = np.ascontiguousarray(
            xt.reshape(4, P, 2, N).transpose(1, 0, 2, 3).astype(NP_FP8))
        in_maps.append(m)
    return in_maps


def kernel(x, qkv_w, qkv_b, out_w, out_b, gate_w, gate_b, scale_gain,
           field_coupling):
    x = np.asarray(x, dtype=np.float32)
    qkv_w = np.asarray(qkv_w, dtype=np.float32)
    qkv_b = np.asarray(qkv_b, dtype=np.float32)
    out_w = np.asarray(out_w, dtype=np.float32)
    out_b = np.asarray(out_b, dtype=np.float32)
    gate_w = np.asarray(gate_w, dtype=np.float32)
    gate_b = np.asarray(gate_b, dtype=np.float32)
    scale_gain = np.asarray(scale_gain, dtype=np.float32)
    field_coupling = np.asarray(field_coupling, dtype=np.float32)

    ob_zero = not np.any(out_b)
    key = ("nc", ob_zero)
    if key not in _CACHE:
        _CACHE[key] = _build_program(ob_zero=ob_zero)
    nc = _CACHE[key]

    shared = _prep_shared(qkv_w, qkv_b, out_w, out_b, gate_w, gate_b,
                          scale_gain, field_coupling)
    in_maps = _make_in_maps(x, shared)

    res = bass_utils.run_bass_kernel_spmd(nc, in_maps, list(range(NCORES)))
    out = np.stack([np.asarray(res.results[b]["y"], dtype=np.float32)
                    for b in range(B)], axis=0)
    return out

